# revision 60
# baseline (speedup 1.0000x reference)
"""Trainium2 Bass kernel for CorpusSupportSets RBF tangent-field.

Math per sample row i (dim 768), one-hot mask selecting dipole k
(z unit-norm so zz == 1):
    t_j  = z . s_j
    m_j  = c_j exp(2 g_j t_j),   c_j = a_j g_j exp(-g_j (1 + ss_j))   (host)
    hn   = -(m0 t0 + m1 t1)
    pn   = m0^2 ss0 + m1^2 ss1 + 2 m0 m1 d - hn^2,   d = s0 . s1     (host)
    r    = exp(-0.5 ln pn)          (ln+exp share one ACT table set)
    out  = (r hn) z + (r m0) s0 + (r m1) s1          (f16, upcast on host)

Device work per 128-row tile: decode the bit-packed one-hot mask to a row
index (max / max_index / max_index-vs-pow2), indirect-gather one f16 table
row [s0|c0 g0 ss0 .|s1|c1 g1 ss1 d], fused multiply-reduce for t0/t1, and a
3-term normalized assembly with work split across Vector and Scalar engines.

Sharding: data-parallel over batch across 8 cores (2048 rows each).
"""
import sys

for _p in ("/opt/trn_rl_repo",):
    if _p not in sys.path:
        sys.path.insert(0, _p)

import numpy as np

import concourse.bass as bass
import concourse.tile as tile
from concourse import mybir
from concourse.bass import IndirectOffsetOnAxis
from concourse.bass_utils import run_bass_kernel_spmd
from concourse.vector_clock import ScopedClock

# ---------------------------------------------------------------------------
# Workaround: this walrus build only accepts ONE semaphore wait per
# instruction; the TileContext exit drain accumulates one wait per live
# semaphore lane.  Split overflow waits onto trailing sync-engine NOPs.
_MAX_WAITS = 1


def _split_waits(nc, inst):
    si = inst.sync_info
    if si is None:
        return
    waits = list(si.on_wait)
    if len(waits) <= _MAX_WAITS:
        return
    inst.sync_info = mybir.SyncInfo(
        on_wait=waits[:_MAX_WAITS], on_update=list(si.on_update)
    )
    for i in range(_MAX_WAITS, len(waits), _MAX_WAITS):
        nop = nc.sync.nop(nofuse=True, hint="drain_wait_overflow")
        nop.ins.sync_info = mybir.SyncInfo(
            on_wait=waits[i : i + _MAX_WAITS], on_update=[]
        )


def _patched_drain_and_barrier(self, tick_clock, wait_clock):
    drain_inst = self.nc.sync.drain()
    wait_clock.add_sem_waits(
        drain_inst.ins, ScopedClock({None: tick_clock.global_clock})
    )
    _split_waits(self.nc, drain_inst.ins)
    self.nc.all_engine_barrier()
    assert self.sems is not None
    popped = self.nc._tile_sem_poison_stack.pop()
    assert popped is self._sem_poison
    self.nc.clear_and_free_semaphores(list(self.sems.allocated().values()))
    self.nc.all_engine_barrier()


_orig_commit = tile.TileContext._commit_instruction


def _patched_commit(self, inst, lazy_reg_writes=True):
    si = getattr(inst, "sync_info", None)
    if (
        si is not None
        and si.on_wait
        and len(si.on_wait) > _MAX_WAITS
        and inst.engine != mybir.EngineType.Unassigned
    ):
        waits = list(si.on_wait)
        inst.sync_info = mybir.SyncInfo(
            on_wait=waits[:_MAX_WAITS], on_update=list(si.on_update)
        )
        for _i, _w in enumerate(waits[_MAX_WAITS:]):
            nop = mybir.InstNoOp(
                name=f"{inst.name}_w{_i}",
                engine=inst.engine,
                sync_info=mybir.SyncInfo(on_wait=[_w], on_update=[]),
                bass_nofuse=True,
            )
            self._add_instruction(nop)
    return _orig_commit(self, inst, lazy_reg_writes)


tile.TileContext._drain_and_barrier = _patched_drain_and_barrier
tile.TileContext._commit_instruction = _patched_commit

# ---------------------------------------------------------------------------
BS, K, DIM = 16384, 1000, 768
NCORES = 8
ROWS = BS // NCORES  # 2048 rows per core
P = 128
NT = ROWS // P  # 16 tiles of 128 rows
GRP = 4  # tiles per group
NG = NT // GRP  # 4 groups
KB = K // 8  # 125 packed-mask bytes per row
HW_ = DIM + 4  # 772: one pole half [s_j | side_j(4)]
TBL_W = 2 * HW_  # 1544 f16 cols per table row
F32 = mybir.dt.float32
F16 = mybir.dt.float16
U8 = mybir.dt.uint8
U32 = mybir.dt.uint32




def build_nc(rows=ROWS):
    NT = rows // P
    OP = mybir.AluOpType
    AT = mybir.ActivationFunctionType
    nc = bass.Bass()
    NT_ = rows // P
    NG_ = NT_ // GRP
    # z/mask/out are pre-shuffled on the host to group-then-partition-major
    # layout so every DMA is 128 large contiguous descriptors.
    zin = nc.dram_tensor("zin", [NG_, P, GRP, DIM], F16, kind="ExternalInput")
    mk = nc.dram_tensor("mk", [P, NT_, KB], U8, kind="ExternalInput")
    tbl = nc.dram_tensor("tbl", [K, TBL_W], F16, kind="ExternalInput")
    pw2in = nc.dram_tensor("pw2", [P, 8], U8, kind="ExternalInput")
    eyein = nc.dram_tensor("eye", [P, P], F16, kind="ExternalInput")
    out = nc.dram_tensor("out", [NG_, P, GRP, DIM], F16, kind="ExternalOutput")

    with tile.TileContext(nc) as tc:
        with (
            tc.tile_pool(name="zp", bufs=4) as zp,
            tc.tile_pool(name="selp", bufs=16) as selp,
            tc.tile_pool(name="outp", bufs=3) as outp,
            tc.tile_pool(name="prodp", bufs=6) as prodp,
            tc.tile_pool(name="scrp", bufs=4) as scrp,
            tc.tile_pool(name="dgp", bufs=12) as dgp,
            tc.tile_pool(name="psp", bufs=4, space="PSUM") as psp,
            tc.tile_pool(name="tiny", bufs=60) as tinyp,
            tc.tile_pool(name="singles", bufs=1) as singles,
        ):
            t0a = singles.tile([P, NT], F32)
            t1a = singles.tile([P, NT], F32)
            m0a = singles.tile([P, NT], F32)
            m1a = singles.tile([P, NT], F32)
            hna = singles.tile([P, NT], F32)
            pna = singles.tile([P, NT], F32)
            lpa = singles.tile([P, NT], F32)
            ra = singles.tile([P, NT], F32)
            rha = singles.tile([P, NT], F32)
            rm0a = singles.tile([P, NT], F32)
            rm1a = singles.tile([P, NT], F32)
            sidef = singles.tile([P, NT, 2, 4], F32)
            bia = singles.tile([P, NT, 8], U32)
            via = singles.tile([P, NT, 8], U32)
            bfa = singles.tile([P, NT], F32)
            vfa = singles.tile([P, NT], F32)
            ixf = singles.tile([P, NT], F32)
            mia = singles.tile([P, NT], U32)
            pw2 = singles.tile([P, 8], U8)
            nc.sync.dma_start(out=pw2[:], in_=pw2in[:])
            mk_a = singles.tile([P, NT, KB], U8)
            nc.sync.dma_start(out=mk_a[:], in_=mk[:])
            zgs = []
            for g in range(NG):
                z_g = zp.tile([P, GRP, DIM], F16, name="z_g", tag="z")
                nc.sync.dma_start(out=z_g[:], in_=zin[g])
                zgs.append(z_g)
            eye = singles.tile([P, P], F16)
            nc.sync.dma_start(out=eye[:], in_=eyein[:])

            def decode_and_gather(g):
                c0, c1 = g * GRP, (g + 1) * GRP
                # packed one-hot -> byte idx B and bit idx v per tile
                for n in range(GRP):
                    j = c0 + n
                    mx = tinyp.tile([P, 8], U8, name="mx", tag="mx")
                    nc.vector.max(out=mx[:], in_=mk_a[:, j, :])
                    nc.vector.max_index(
                        out=bia[:, j, :], in_max=mx[:], in_values=mk_a[:, j, :]
                    )
                    nc.vector.max_index(
                        out=via[:, j, :], in_max=mx[:], in_values=pw2[:]
                    )
                # idx = 8*B + v  (batched; u32->f32 casts on DVE)
                nc.vector.tensor_copy(out=bfa[:, c0:c1], in_=bia[:, c0:c1, 0])
                nc.vector.tensor_copy(out=vfa[:, c0:c1], in_=via[:, c0:c1, 0])
                nc.vector.scalar_tensor_tensor(
                    out=ixf[:, c0:c1], in0=bfa[:, c0:c1], scalar=8.0,
                    in1=vfa[:, c0:c1], op0=OP.mult, op1=OP.add,
                )
                nc.vector.tensor_copy(out=mia[:, c0:c1], in_=ixf[:, c0:c1])
                sels = []
                for n in range(GRP):
                    j = c0 + n
                    sel = selp.tile([P, TBL_W], F16, name="sel", tag="sel")
                    nc.gpsimd.indirect_dma_start(
                        out=sel[:],
                        out_offset=None,
                        in_=tbl[:],
                        in_offset=IndirectOffsetOnAxis(
                            ap=mia[:, j : j + 1], axis=0
                        ),
                    )
                    sels.append(sel)
                return sels

            def tphase(g, sels):
                c0, c1 = g * GRP, (g + 1) * GRP
                z_g = zgs[g]
                for n in range(GRP):
                    j = c0 + n
                    if n == GRP - 1:
                        # last tile of each group: DVE fused multiply-reduce,
                        # so ACT's accum queue drains one tile earlier
                        s0cr = scrp.tile([P, DIM], F16, name="s0cr", tag="scr")
                        nc.vector.scalar_tensor_tensor(
                            out=s0cr[:], in0=z_g[:, n, :], scalar=1.0,
                            in1=sels[n][:, 0:DIM], op0=OP.mult, op1=OP.mult,
                            accum_out=t0a[:, j : j + 1],
                        )
                        s1cr = scrp.tile([P, DIM], F16, name="s1cr", tag="scr")
                        nc.vector.scalar_tensor_tensor(
                            out=s1cr[:], in0=z_g[:, n, :], scalar=1.0,
                            in1=sels[n][:, HW_ : HW_ + DIM], op0=OP.mult,
                            op1=OP.mult, accum_out=t1a[:, j : j + 1],
                        )
                        continue
                    prod = prodp.tile([P, 2, DIM], F16, name="prod", tag="prod")
                    zv = z_g[:, n : n + 1, :].broadcast_to([P, 2, DIM])
                    s2v = sels[n][:].rearrange("p (a w) -> p a w", a=2)[
                        :, :, 0:DIM
                    ]
                    nc.vector.tensor_tensor(out=prod[:], in0=zv, in1=s2v, op=OP.mult)
                    junk0 = scrp.tile([P, DIM], F16, name="junk0", tag="scr")
                    nc.scalar.activation(
                        out=junk0[:], in_=prod[:, 0, :], func=AT.Copy,
                        accum_out=t0a[:, j : j + 1],
                    )
                    junk1 = scrp.tile([P, DIM], F16, name="junk1", tag="scr")
                    nc.scalar.activation(
                        out=junk1[:], in_=prod[:, 1, :], func=AT.Copy,
                        accum_out=t1a[:, j : j + 1],
                    )
                # upcast side constants [c, g, ss | c, g, ss/d] (DVE casts)
                for n in range(GRP):
                    j = c0 + n
                    sv = sels[n][:].rearrange("p (a w) -> p a w", a=2)[:, :, DIM:]
                    nc.vector.tensor_copy(out=sidef[:, j, :, :], in_=sv)

            def smalls(c0, c1):
                """Scalar math for tile columns [c0, c1)."""
                OPm, OPa, OPs = OP.mult, OP.add, OP.subtract
                NC_ = c1 - c0
                cs0 = sidef[:, c0:c1, 0, 0]
                gs0 = sidef[:, c0:c1, 0, 1]
                ssa = sidef[:, c0:c1, 0, 2]
                cs1 = sidef[:, c0:c1, 1, 0]
                gs1 = sidef[:, c0:c1, 1, 1]
                ssb = sidef[:, c0:c1, 1, 2]
                dd = sidef[:, c0:c1, 1, 3]
                t0 = t0a[:, c0:c1]
                t1 = t1a[:, c0:c1]

                def T(nm):
                    return tinyp.tile([P, NC_], F32, name=nm, tag="tiny")

                gt0, gt1, e0, e1 = T("gt0"), T("gt1"), T("e0"), T("e1")
                nc.vector.tensor_tensor(out=gt0[:], in0=t0, in1=gs0, op=OPm)
                nc.vector.tensor_tensor(out=gt1[:], in0=t1, in1=gs1, op=OPm)
                nc.scalar.activation(out=e0[:], in_=gt0[:], func=AT.Exp, scale=2.0)
                nc.scalar.activation(out=e1[:], in_=gt1[:], func=AT.Exp, scale=2.0)
                nc.vector.tensor_tensor(out=m0a[:, c0:c1], in0=e0[:], in1=cs0, op=OPm)
                nc.vector.tensor_tensor(out=m1a[:, c0:c1], in0=e1[:], in1=cs1, op=OPm)
                u0, u1 = T("u0"), T("u1")
                nc.vector.tensor_tensor(out=u0[:], in0=m0a[:, c0:c1], in1=t0, op=OPm)
                nc.vector.tensor_tensor(out=u1[:], in0=m1a[:, c0:c1], in1=t1, op=OPm)
                nc.vector.scalar_tensor_tensor(
                    out=hna[:, c0:c1], in0=u0[:], scalar=-1.0, in1=u1[:],
                    op0=OPm, op1=OPs,
                )
                v0, v1, mm, hh, w = T("v0"), T("v1"), T("mm"), T("hh"), T("w")
                nc.vector.tensor_tensor(out=v0[:], in0=m0a[:, c0:c1], in1=m0a[:, c0:c1], op=OPm)
                nc.vector.tensor_tensor(out=v0[:], in0=v0[:], in1=ssa, op=OPm)
                nc.vector.tensor_tensor(out=v1[:], in0=m1a[:, c0:c1], in1=m1a[:, c0:c1], op=OPm)
                nc.vector.tensor_tensor(out=v1[:], in0=v1[:], in1=ssb, op=OPm)
                nc.vector.tensor_tensor(out=mm[:], in0=m0a[:, c0:c1], in1=m1a[:, c0:c1], op=OPm)
                nc.vector.tensor_tensor(out=mm[:], in0=mm[:], in1=dd, op=OPm)
                nc.vector.tensor_tensor(out=hh[:], in0=hna[:, c0:c1], in1=hna[:, c0:c1], op=OPm)
                nc.vector.tensor_tensor(out=w[:], in0=v0[:], in1=v1[:], op=OPa)
                nc.vector.scalar_tensor_tensor(
                    out=w[:], in0=mm[:], scalar=2.0, in1=w[:], op0=OPm, op1=OPa
                )
                nc.vector.tensor_tensor(out=pna[:, c0:c1], in0=w[:], in1=hh[:], op=OPs)
                # r = exp(-0.5 ln pn)   (same ACT table set as Exp)
                nc.scalar.activation(out=lpa[:, c0:c1], in_=pna[:, c0:c1], func=AT.Ln)
                nc.scalar.activation(out=ra[:, c0:c1], in_=lpa[:, c0:c1], func=AT.Exp, scale=-0.5)
                nc.vector.tensor_tensor(out=rha[:, c0:c1], in0=ra[:, c0:c1], in1=hna[:, c0:c1], op=OPm)
                nc.vector.tensor_tensor(out=rm0a[:, c0:c1], in0=ra[:, c0:c1], in1=m0a[:, c0:c1], op=OPm)
                nc.vector.tensor_tensor(out=rm1a[:, c0:c1], in0=ra[:, c0:c1], in1=m1a[:, c0:c1], op=OPm)

            def phase2(g, sels):
                c0 = g * GRP
                z_g = zgs[g]
                pg = outp.tile([P, GRP, DIM], F16, name="pg", tag="pg")
                # stage 1: ALL diag builds (DVE) so PE never waits mid-group
                diags = []
                for n in range(GRP):
                    j = c0 + n
                    z_on_pe = True
                    d0 = dgp.tile([P, P], F16, name="d0", tag="dg")
                    nc.vector.tensor_scalar(
                        out=d0[:], in0=eye[:], scalar1=rm0a[:, j : j + 1],
                        scalar2=None, op0=OP.mult,
                    )
                    d1 = dgp.tile([P, P], F16, name="d1", tag="dg")
                    nc.vector.tensor_scalar(
                        out=d1[:], in0=eye[:], scalar1=rm1a[:, j : j + 1],
                        scalar2=None, op0=OP.mult,
                    )
                    dz = None
                    if z_on_pe:
                        dz = dgp.tile([P, P], F16, name="dz", tag="dg")
                        nc.vector.tensor_scalar(
                            out=dz[:], in0=eye[:], scalar1=rha[:, j : j + 1],
                            scalar2=None, op0=OP.mult,
                        )
                    diags.append((d0, d1, dz))
                # stage 2: all PE matmuls into PSUM
                pss = []
                for n in range(GRP):
                    j = c0 + n
                    d0, d1, dz = diags[n]
                    s0v = sels[n][:, 0:DIM]
                    s1v = sels[n][:, HW_ : HW_ + DIM]
                    z_n = z_g[:, n, :]
                    z_on_pe = dz is not None
                    ps = psp.tile([P, DIM], F32, name="ps", tag="ps")
                    for lo, hi in ((0, 512), (512, DIM)):
                        nc.tensor.matmul(
                            ps[:, lo:hi], d0[:], s0v[:, lo:hi],
                            start=True, stop=False,
                        )
                        nc.tensor.matmul(
                            ps[:, lo:hi], d1[:], s1v[:, lo:hi],
                            start=False, stop=not z_on_pe,
                        )
                        if z_on_pe:
                            nc.tensor.matmul(
                                ps[:, lo:hi], dz[:], z_n[:, lo:hi],
                                start=False, stop=True,
                            )
                    pss.append(ps)
                # stage 3: finals (ACT cast / fused DVE stt) then store
                for n in range(GRP):
                    j = c0 + n
                    p_n = pg[:, n, :]
                    if diags[n][2] is not None:
                        nc.scalar.activation(out=p_n, in_=pss[n][:], func=AT.Copy)
                    else:
                        nc.vector.scalar_tensor_tensor(
                            out=p_n, in0=z_g[:, n, :], scalar=rha[:, j : j + 1],
                            in1=pss[n][:], op0=OP.mult, op1=OP.add,
                        )
                nc.sync.dma_start(out=out[g], in_=pg[:])

            # interleaved pipeline: decode feeds gathers early, then groups
            # flow with per-group smalls so assembly streams group by group
            sels = [None] * NG
            sels[0] = decode_and_gather(0)
            sels[1] = decode_and_gather(1)
            tphase(0, sels[0])
            sels[2] = decode_and_gather(2)
            smalls(0, GRP)
            tphase(1, sels[1])
            sels[3] = decode_and_gather(3)
            smalls(GRP, 2 * GRP)
            phase2(0, sels[0])
            tphase(2, sels[2])
            smalls(2 * GRP, 3 * GRP)
            phase2(1, sels[1])
            tphase(3, sels[3])
            phase2(2, sels[2])
            smalls(3 * GRP, 4 * GRP)
            phase2(3, sels[3])
    return nc


_NC_CACHE = None


def _get_nc():
    global _NC_CACHE
    if _NC_CACHE is None:
        _NC_CACHE = build_nc()
    return _NC_CACHE


def build_in_maps(inputs):
    z = np.asarray(inputs["z"], dtype=np.float32).astype(np.float16)
    mask = np.asarray(inputs["support_sets_mask"])
    mask_bits = np.packbits(mask != 0, axis=1)  # [BS, 125], big-endian bits
    SS = np.asarray(inputs["SUPPORT_SETS"], dtype=np.float32)
    AL = np.asarray(inputs["ALPHAS"], dtype=np.float32)
    LG = np.asarray(inputs["LOGGAMMA"], dtype=np.float32)

    s_f = SS.astype(np.float16)  # device sees f16 s; constants derive from it
    s0 = s_f[:, :DIM].astype(np.float32)
    s1 = s_f[:, DIM:].astype(np.float32)
    g = np.exp(LG)
    ss0 = (s0 * s0).sum(1)
    ss1 = (s1 * s1).sum(1)
    d = (s0 * s1).sum(1)
    c = AL * g * np.exp(-g * (1.0 + np.stack([ss0, ss1], 1)))
    tbl = np.zeros((K, TBL_W), dtype=np.float16)
    tbl[:, 0:DIM] = s_f[:, :DIM]
    tbl[:, DIM + 0] = c[:, 0]
    tbl[:, DIM + 1] = g[:, 0]
    tbl[:, DIM + 2] = ss0
    tbl[:, HW_ : HW_ + DIM] = s_f[:, DIM:]
    tbl[:, HW_ + DIM + 0] = c[:, 1]
    tbl[:, HW_ + DIM + 1] = g[:, 1]
    tbl[:, HW_ + DIM + 2] = ss1
    tbl[:, HW_ + DIM + 3] = d
    tbl = np.ascontiguousarray(tbl)
    pw2 = np.tile(
        np.array([128, 64, 32, 16, 8, 4, 2, 1], np.uint8), (P, 1)
    )
    eye = np.eye(P, dtype=np.float16)

    def shuf_pm(a):
        # [ROWS, W] -> partition-major [P, NT, W]
        return np.ascontiguousarray(
            a.reshape(NT, P, a.shape[1]).transpose(1, 0, 2)
        )

    def shuf_g(a):
        # [ROWS, W] -> group-major [NG, P, GRP, W]
        return np.ascontiguousarray(
            a.reshape(NG, GRP, P, a.shape[1]).transpose(0, 2, 1, 3)
        )

    return [
        {
            "zin": shuf_g(z[c_ * ROWS : (c_ + 1) * ROWS]),
            "mk": shuf_pm(mask_bits[c_ * ROWS : (c_ + 1) * ROWS]),
            "tbl": tbl,
            "pw2": pw2,
            "eye": eye,
        }
        for c_ in range(NCORES)
    ]


def kernel(support_sets_mask, z, SUPPORT_SETS, ALPHAS, LOGGAMMA):
    in_maps = build_in_maps(
        dict(
            support_sets_mask=support_sets_mask, z=z,
            SUPPORT_SETS=SUPPORT_SETS, ALPHAS=ALPHAS, LOGGAMMA=LOGGAMMA,
        )
    )
    nc = _get_nc()
    res = run_bass_kernel_spmd(nc, in_maps, list(range(NCORES)))
    # device output is [NG, P, GRP, DIM]; unshuffle to [ROWS, DIM]
    return np.concatenate(
        [
            res.results[c]["out"].transpose(0, 2, 1, 3).reshape(ROWS, DIM)
            for c in range(NCORES)
        ],
        axis=0,
    ).astype(np.float32)
